# revision 7
# baseline (speedup 1.0000x reference)
"""Trainium2 Bass kernel for MessagePassingPC (predictive-coding GNN message passing).

Algorithm (reference):
    A = adj>0;  n_par = colsum(A);  n_chi = rowsum(A)
    mu0 = obs * mask
    repeat 8x:
        pred   = tanh(per-node GEMV(mu, W) + b)          # W[n] is a 128x128 per-node matrix
        x_pred = (A.T @ pred) / max(n_par,1)             # zero column-sum handles n_par==0
        eps    = mu - x_pred                             # mask term is a no-op (mu==obs on masked)
        corr   = (A @ eps) / max(n_chi,1)
        mu    += 0.1*(1-mask)*(-eps + 0.5*corr)          # masked nodes pinned to obs
    outputs: mu, eps(last), fe = 0.5*sum(eps^2)

Distribution: nodes sharded 512/core over 8 cores. Per step, each core:
  - streams its W shard (bf16) from HBM and runs 512 single-column PE matmuls
    (stationary = W[n].T, moving = muT column) accumulating predT (feat x node) in PSUM,
  - adds b, applies tanh (ACT), transposes its shard to node-major, AllGathers pred,
  - aggregates with resident bf16 adjacency slices (A[:,shard] and A[shard,:].T) as
    32-K-tile PE matmuls with N=512,
  - scales by 1/deg with exact fp32 replicated-constant tiles, updates mu in fp32.
All matmul inputs bf16 (fp32 PSUM accumulate); mu/eps state kept fp32.
"""

import numpy as np
import ml_dtypes

N = 4096
D = 128
NCORES = 8
S = N // NCORES          # 512 nodes per core
KT = N // 128            # 32 contraction tiles
MT = S // 128            # 4 row tiles per shard
GN = 32                  # nodes per W chunk (1 MiB bf16 chunks)
NG = S // GN             # 16 chunks per step
STEPS = 8
ETA = 0.1

_BF16 = ml_dtypes.bfloat16
_MODULE = None


def _build_module():
    import concourse.mybir as mybir
    from concourse import bacc
    from concourse.tile import TileContext

    bf = mybir.dt.bfloat16
    f32 = mybir.dt.float32
    AF = mybir.ActivationFunctionType

    nc = bacc.Bacc(None, target_bir_lowering=False, num_devices=NCORES)

    wt_d = nc.dram_tensor("wt", [NG, 128, GN * 128], bf, kind="ExternalInput")
    acol_d = nc.dram_tensor("acol", [KT, 128, S], bf, kind="ExternalInput")
    arowt_d = nc.dram_tensor("arowt", [KT, 128, S], bf, kind="ExternalInput")
    bt_d = nc.dram_tensor("bt", [128, S], f32, kind="ExternalInput")
    mut0_d = nc.dram_tensor("mut0", [128, S], f32, kind="ExternalInput")
    pscalet_d = nc.dram_tensor("pscalet", [128, S], f32, kind="ExternalInput")
    e1t_d = nc.dram_tensor("e1t", [128, S], f32, kind="ExternalInput")
    e2t_d = nc.dram_tensor("e2t", [128, S], f32, kind="ExternalInput")
    identbf_d = nc.dram_tensor("identbf", [128, 128], bf, kind="ExternalInput")
    identf_d = nc.dram_tensor("identf", [128, 128], f32, kind="ExternalInput")

    mu_out_d = nc.dram_tensor("mu_out", [S, 128], f32, kind="ExternalOutput")
    eps_out_d = nc.dram_tensor("eps_out", [S, 128], f32, kind="ExternalOutput")

    rg = [list(range(NCORES))]

    with TileContext(nc) as tc:
        with (
            tc.tile_pool(name="const", bufs=1) as constp,
            tc.tile_pool(name="state", bufs=1) as statep,
            tc.tile_pool(name="wpool", bufs=8) as wpool,
            tc.tile_pool(name="lhsp", bufs=8) as lhsp,
            tc.tile_pool(name="work", bufs=2) as workp,
            tc.tile_pool(name="ship", bufs=4) as shipp,
            tc.tile_pool(name="psacc", bufs=1, space="PSUM") as psacc,
            tc.tile_pool(name="pstp", bufs=2, space="PSUM") as pstp,
            tc.tile_pool(name="dram", bufs=1, space="DRAM") as dramp,
        ):
            # ---- resident constants ----
            ident_bf = constp.tile([128, 128], bf, name="ident_bf")
            nc.sync.dma_start(ident_bf[:], identbf_d[:])
            ident_f = constp.tile([128, 128], f32, name="ident_f")
            nc.sync.dma_start(ident_f[:], identf_d[:])
            bt_sb = constp.tile([128, S], f32, name="bt_sb")
            nc.sync.dma_start(bt_sb[:], bt_d[:])
            pscalet_sb = constp.tile([128, S], f32, name="pscalet_sb")
            nc.sync.dma_start(pscalet_sb[:], pscalet_d[:])
            e1t_sb = constp.tile([128, S], f32, name="e1t_sb")
            nc.sync.dma_start(e1t_sb[:], e1t_d[:])
            e2t_sb = constp.tile([128, S], f32, name="e2t_sb")
            nc.sync.dma_start(e2t_sb[:], e2t_d[:])
            acol_sb = []
            arowt_sb = []
            for k in range(KT):
                a = constp.tile([128, S], bf, name=f"acol_sb{k}")
                nc.sync.dma_start(a[:], acol_d[k])
                acol_sb.append(a)
                r = constp.tile([128, S], bf, name=f"arowt_sb{k}")
                nc.sync.dma_start(r[:], arowt_d[k])
                arowt_sb.append(r)

            # ---- state ----
            mut = statep.tile([128, S], f32, name="mut")
            nc.sync.dma_start(mut[:], mut0_d[:])
            mut_bf = statep.tile([128, S], bf, name="mut_bf")
            nc.vector.tensor_copy(mut_bf[:], mut[:])
            epst = statep.tile([128, S], f32, name="epst")

            for s in range(STEPS):
                # per-step collective buffers (Shared outputs are single-write)
                pred_bounce = dramp.tile([S, 128], bf, name=f"pred_bounce{s}")
                pred_full = dramp.tile([N, 128], bf, name=f"pred_full{s}", addr_space="Shared")
                eps_bounce = dramp.tile([S, 128], bf, name=f"eps_bounce{s}")
                eps_full = dramp.tile([N, 128], bf, name=f"eps_full{s}", addr_space="Shared")
                # --- phase A: per-node GEMV, predT[o, n] accumulated in PSUM ---
                predt_ps = psacc.tile([128, S], f32, tag="predt_ps", name=f"predt_ps{s}")
                for g in range(NG):
                    wtile = wpool.tile([128, GN * 128], bf, tag="wtile", name=f"w{s}_{g}")
                    nc.gpsimd.dma_start(wtile[:], wt_d[g])
                    for ln in range(GN):
                        n = g * GN + ln
                        nc.tensor.matmul(
                            predt_ps[:, n : n + 1],
                            wtile[:, ln * 128 : (ln + 1) * 128],
                            mut_bf[:, n : n + 1],
                            start=True,
                            stop=True,
                        )

                # --- phase B: + b, tanh ---
                ztmp = workp.tile([128, S], f32, tag="ztmp", name=f"ztmp{s}")
                nc.vector.tensor_add(ztmp[:], predt_ps[:], bt_sb[:])
                predt_bf = workp.tile([128, S], bf, tag="predt_bf", name=f"predt_bf{s}")
                nc.scalar.activation(predt_bf[:], ztmp[:], AF.Tanh)

                # --- phase C: transpose shard to node-major, ship, AllGather ---
                for t in range(MT):
                    tp_ps = pstp.tile([128, 128], bf, tag="tp_bf", name=f"ptp{s}_{t}")
                    nc.tensor.transpose(
                        tp_ps[:], predt_bf[:, t * 128 : (t + 1) * 128], ident_bf[:]
                    )
                    pnt = shipp.tile([128, 128], bf, tag="pnt", name=f"pnt{s}_{t}")
                    nc.vector.tensor_copy(pnt[:], tp_ps[:])
                    nc.sync.dma_start(pred_bounce[t * 128 : (t + 1) * 128, :], pnt[:])
                nc.gpsimd.collective_compute(
                    "AllGather",
                    mybir.AluOpType.bypass,
                    replica_groups=rg,
                    ins=[pred_bounce[:]],
                    outs=[pred_full[:]],
                )

                # --- phase D: parent aggregation  x_predT = sum_k pred_k^T-tiles @ Acol_k ---
                xpt_ps = psacc.tile([128, S], f32, tag="xpt_ps", name=f"xpt_ps{s}")
                for k in range(KT):
                    pf = lhsp.tile([128, 128], bf, tag="pf", name=f"pf{s}_{k}")
                    nc.sync.dma_start(pf[:], pred_full[k * 128 : (k + 1) * 128, :])
                    nc.tensor.matmul(
                        xpt_ps[:],
                        pf[:],
                        acol_sb[k][:],
                        start=(k == 0),
                        stop=(k == KT - 1),
                    )

                # --- phase E: epsT = muT - pscaleT * x_predT ---
                xsc = workp.tile([128, S], f32, tag="xsc", name=f"xsc{s}")
                nc.vector.tensor_mul(xsc[:], xpt_ps[:], pscalet_sb[:])
                nc.vector.tensor_sub(epst[:], mut[:], xsc[:])

                # --- phase F: ship eps (transpose + cast bf16), AllGather ---
                for t in range(MT):
                    tpf_ps = pstp.tile([128, 128], f32, tag="tp_f32", name=f"etp{s}_{t}")
                    nc.tensor.transpose(
                        tpf_ps[:], epst[:, t * 128 : (t + 1) * 128], ident_f[:]
                    )
                    ent = shipp.tile([128, 128], bf, tag="ent", name=f"ent{s}_{t}")
                    nc.vector.tensor_copy(ent[:], tpf_ps[:])
                    nc.sync.dma_start(eps_bounce[t * 128 : (t + 1) * 128, :], ent[:])
                nc.gpsimd.collective_compute(
                    "AllGather",
                    mybir.AluOpType.bypass,
                    replica_groups=rg,
                    ins=[eps_bounce[:]],
                    outs=[eps_full[:]],
                )

                # --- phase G: child aggregation  corrT = sum_k eps_k^T-tiles @ ArowT_k ---
                corrt_ps = psacc.tile([128, S], f32, tag="corrt_ps", name=f"corrt_ps{s}")
                for k in range(KT):
                    ef = lhsp.tile([128, 128], bf, tag="ef", name=f"ef{s}_{k}")
                    nc.sync.dma_start(ef[:], eps_full[k * 128 : (k + 1) * 128, :])
                    nc.tensor.matmul(
                        corrt_ps[:],
                        ef[:],
                        arowt_sb[k][:],
                        start=(k == 0),
                        stop=(k == KT - 1),
                    )

                # --- phase H: mu update (fp32) ---
                t1 = workp.tile([128, S], f32, tag="t1", name=f"t1_{s}")
                nc.vector.tensor_mul(t1[:], corrt_ps[:], e2t_sb[:])
                t2 = workp.tile([128, S], f32, tag="t2", name=f"t2_{s}")
                nc.vector.tensor_mul(t2[:], epst[:], e1t_sb[:])
                t3 = workp.tile([128, S], f32, tag="t3", name=f"t3_{s}")
                nc.vector.tensor_sub(t3[:], t1[:], t2[:])
                nc.vector.tensor_add(mut[:], mut[:], t3[:])
                nc.vector.tensor_copy(mut_bf[:], mut[:])

            # ---- outputs: transpose muT/epsT back to node-major fp32 ----
            for t in range(MT):
                ops = pstp.tile([128, 128], f32, tag="tp_f32", name=f"otp_mu{t}")
                nc.tensor.transpose(ops[:], mut[:, t * 128 : (t + 1) * 128], ident_f[:])
                osb = shipp.tile([128, 128], f32, tag="osb", name=f"osb_mu{t}")
                nc.vector.tensor_copy(osb[:], ops[:])
                nc.sync.dma_start(mu_out_d[t * 128 : (t + 1) * 128, :], osb[:])
            for t in range(MT):
                ope = pstp.tile([128, 128], f32, tag="tp_f32", name=f"otp_eps{t}")
                nc.tensor.transpose(ope[:], epst[:, t * 128 : (t + 1) * 128], ident_f[:])
                oeb = shipp.tile([128, 128], f32, tag="oeb", name=f"oeb_eps{t}")
                nc.vector.tensor_copy(oeb[:], ope[:])
                nc.sync.dma_start(eps_out_d[t * 128 : (t + 1) * 128, :], oeb[:])

    nc.compile()
    return nc


def _get_module():
    global _MODULE
    if _MODULE is None:
        _MODULE = _build_module()
    return _MODULE


def kernel(observations, adj_matrix, obs_mask, W, b):
    obs = np.asarray(observations, dtype=np.float32)
    A = (np.asarray(adj_matrix) > 0).astype(np.float32)
    mask = np.asarray(obs_mask)
    Wf = np.asarray(W, dtype=np.float32)
    bf_ = np.asarray(b, dtype=np.float32)

    n_par = A.sum(axis=0)
    n_chi = A.sum(axis=1)
    parscale = (1.0 / np.maximum(n_par, 1.0)).astype(np.float32)
    chiscale = (1.0 / np.maximum(n_chi, 1.0)).astype(np.float32)
    maskf = mask.astype(np.float32)
    mu0 = obs * maskf[:, None]
    e1 = (ETA * (1.0 - maskf)).astype(np.float32)
    e2 = (0.5 * ETA * (1.0 - maskf) * chiscale).astype(np.float32)

    At = np.ascontiguousarray(A.T)

    in_maps = []
    for c in range(NCORES):
        sl = slice(c * S, (c + 1) * S)
        # wt[g, d, ln*128+o] = W[g*GN+ln, o, d]
        wt = np.ascontiguousarray(
            Wf[sl].reshape(NG, GN, D, D).transpose(0, 3, 1, 2)
        ).reshape(NG, D, GN * D).astype(_BF16)
        acol = A[:, sl].reshape(KT, 128, S).astype(_BF16)
        arowt = At[:, sl].reshape(KT, 128, S).astype(_BF16)
        rep = np.ones((D, 1), np.float32)
        in_maps.append(
            {
                "wt": wt,
                "acol": np.ascontiguousarray(acol),
                "arowt": np.ascontiguousarray(arowt),
                "bt": np.ascontiguousarray(bf_[sl].T),
                "mut0": np.ascontiguousarray(mu0[sl].T),
                "pscalet": np.ascontiguousarray(rep * parscale[sl][None, :]),
                "e1t": np.ascontiguousarray(rep * e1[sl][None, :]),
                "e2t": np.ascontiguousarray(rep * e2[sl][None, :]),
                "identbf": np.eye(128, dtype=_BF16),
                "identf": np.eye(128, dtype=np.float32),
            }
        )

    from concourse.bass_utils import run_bass_kernel_spmd

    nc = _get_module()
    res = run_bass_kernel_spmd(nc, in_maps, core_ids=list(range(NCORES)))

    mu = np.concatenate([r["mu_out"] for r in res.results], axis=0)
    eps = np.concatenate([r["eps_out"] for r in res.results], axis=0)
    fe = np.float32(0.5 * np.sum(eps.astype(np.float64) ** 2))
    return mu, eps, fe
